# revision 13
# baseline (speedup 1.0000x reference)
"""DeepFM forward kernel for 8 Trainium2 NeuronCores.

Strategy: data-parallel over the batch (2048 samples/core). The gather is
Pool-engine descriptor-generation bound (~8.5ns/row on this runtime), so the
kernel minimizes Pool instruction count: the embedding tables are bf16-packed
4 vocab entries per 256-byte row so the packed row index (v>>2) fits
dma_gather's 15-bit index format, and one bulk dma_gather per (field,
half-batch) — 78 instructions of 1024 rows — replaces 624 per-tile indirect
DMAs. The (v&3) sub-row select folds into the Xv scaling: the host pre-splits
Xv into 4 slot-masked weight tensors and DVE blends the 4 candidate sub-rows
per 4-field chunk. Tiles 0-7 are normalized while the second half-batch still
gathers. The L2 field-normalization runs sample-major on DVE, 5 bf16 PE
transposes per 128-sample tile produce the feature-major MLP input, and the
3-layer MLP with training-mode BatchNorm (cross-core AllReduce on CC cores)
runs feature-major.
"""

import numpy as np
import ml_dtypes

import concourse.bass as bass
import concourse.mybir as mybir
import concourse.tile as tile
from concourse import bacc
from concourse.bass_utils import run_bass_kernel_spmd

N, F, V, E = 16384, 39, 100000, 16
W17 = E + 1            # payload row width (16 emb2 cols + 1 emb1 col)
PK = 4                 # vocab entries packed per gather row
RW = 128               # gather row width in bf16 elems (256 B)
NPR = V // PK          # 25000 packed rows per field
FE = F * E             # 624
HID = 400
BN_EPS = 1e-5
NC = 8
NLOC = N // NC         # 2048
P = 128
NT = NLOC // P         # 16 tiles of 128 samples
S = 512                # sample group width for the MLP
NG = NLOC // S         # 4 groups
TPG = S // P           # tiles per group
STRIP = NT * W17       # 272 cols per field strip in xfall
HW_NIDX = 1024         # >1024 idxs per dma_gather faults on this HW
NH = NLOC // HW_NIDX   # 2 half-batches
TPH = NT // NH         # 8 tiles per half
CHUNKS = [4] * 9 + [3] # field chunks for gather+blend

KC = [128, 128, 128, 128, 112]          # K blocks of FE=624
KO = [0, 128, 256, 384, 512]
MB = [128, 128, 128, 16]                # blocks of HID=400
MO = [0, 128, 256, 384]

F32 = mybir.dt.float32
BF16 = mybir.dt.bfloat16
I16 = mybir.dt.int16
AF = mybir.ActivationFunctionType
MUL = mybir.AluOpType.mult
ADD = mybir.AluOpType.add
SUB = mybir.AluOpType.subtract


def build_kernel(n_cores=NC, mlp_dt=BF16):
    nc = bacc.Bacc("TRN2", target_bir_lowering=False, debug=False,
                   num_devices=n_cores)

    dram = {}
    def din(name, shape, dt):
        dram[name] = nc.dram_tensor(name, shape, dt, kind="ExternalInput").ap()
        return dram[name]

    embp = din("embp", [F * NPR, RW], BF16)
    idx16 = din("idx16", [P, F * P], I16)   # [p, f*128+c] = (Xi[c*16+p%16, f]>>2)
    wv = din("wv", [P, 4 * FE], BF16)       # [p, k*624+f*16+t] = Xv*(slot==k)
    w1t = din("w1t", [FE, HID], BF16)
    w2t = din("w2t", [HID, HID], BF16)
    w3t = din("w3t", [HID, HID], BF16)
    gg = [din(f"g{i}", [HID, 1], F32) for i in (1, 2, 3)]
    bb = [din(f"bt{i}", [HID, 1], F32) for i in (1, 2, 3)]
    bias = din("bias", [1, 1], F32)
    identb = din("identb", [P, P], BF16)
    onesb = din("onesb", [P, 1], BF16)
    out = nc.dram_tensor("out", [1, NLOC], F32, kind="ExternalOutput").ap()

    with tile.TileContext(nc) as tc:
        import contextlib
        with contextlib.ExitStack() as ctx:
            pers = ctx.enter_context(tc.tile_pool(name="pers", bufs=1))
            gp = ctx.enter_context(tc.tile_pool(name="gp", bufs=3))
            bp = ctx.enter_context(tc.tile_pool(name="bp", bufs=2))
            xsp = ctx.enter_context(tc.tile_pool(name="xsp", bufs=3))
            nrm = ctx.enter_context(tc.tile_pool(name="nrm", bufs=3))
            hp = ctx.enter_context(tc.tile_pool(name="hp", bufs=3))
            actp = ctx.enter_context(tc.tile_pool(name="actp", bufs=2))
            osb = ctx.enter_context(tc.tile_pool(name="osb", bufs=2))
            stp = ctx.enter_context(tc.tile_pool(name="stp", bufs=1))
            dramp = ctx.enter_context(tc.tile_pool(name="dramp", bufs=1, space="DRAM"))
            zp = ctx.enter_context(tc.tile_pool(name="zp", bufs=2))

            ps_t = ctx.enter_context(tc.tile_pool(name="ps_t", bufs=2, space="PSUM"))
            ps_po = ctx.enter_context(tc.tile_pool(name="ps_po", bufs=1, space="PSUM"))
            ps_z = ctx.enter_context(tc.tile_pool(name="ps_z", bufs=4, space="PSUM"))

            # ---- constants / weights (HWDGE loads only) ------------------
            ident_t = pers.tile([P, P], mlp_dt, tag="identb", name="identb")
            nc.sync.dma_start(out=ident_t[:], in_=identb[:])
            ones_t = pers.tile([P, 1], mlp_dt, tag="onesb", name="onesb")
            nc.sync.dma_start(out=ones_t[:], in_=onesb[:])
            bias_t = pers.tile([1, 1], F32, tag="bias", name="bias")
            nc.sync.dma_start(out=bias_t[:], in_=bias[:])
            eps_t = pers.tile([P, 1], F32, tag="eps", name="eps")
            nc.vector.memset(eps_t[:], BN_EPS)
            idx_t = pers.tile([P, F * P], I16, tag="idx16", name="idx16")
            nc.sync.dma_start(out=idx_t[:], in_=idx16[:])
            wv_t = pers.tile([P, 4 * FE], BF16, tag="wv", name="wv")
            nc.sync.dma_start(out=wv_t[:], in_=wv[:])

            w1sb = []
            for c in range(5):
                t = pers.tile([KC[c], HID], mlp_dt, tag=f"w1c{c}", name=f"w1c{c}")
                nc.sync.dma_start(out=t[:], in_=w1t[KO[c]:KO[c] + KC[c], :])
                w1sb.append(t)
            w2sb, w3sb = [], []
            for k in range(4):
                t = pers.tile([MB[k], HID], mlp_dt, tag=f"w2c{k}", name=f"w2c{k}")
                nc.sync.dma_start(out=t[:], in_=w2t[MO[k]:MO[k] + MB[k], :])
                w2sb.append(t)
                t = pers.tile([MB[k], HID], mlp_dt, tag=f"w3c{k}", name=f"w3c{k}")
                nc.sync.dma_start(out=t[:], in_=w3t[MO[k]:MO[k] + MB[k], :])
                w3sb.append(t)
            g_sb = [[pers.tile([MB[m], 1], F32, tag=f"g{l}m{m}", name=f"g{l}m{m}") for m in range(4)]
                    for l in range(3)]
            bt_sb = [[pers.tile([MB[m], 1], F32, tag=f"bt{l}m{m}", name=f"bt{l}m{m}") for m in range(4)]
                     for l in range(3)]
            for l in range(3):
                for m in range(4):
                    nc.sync.dma_start(out=g_sb[l][m][:], in_=gg[l][MO[m]:MO[m] + MB[m], :])
                    nc.sync.dma_start(out=bt_sb[l][m][:], in_=bb[l][MO[m]:MO[m] + MB[m], :])

            # persistent activations
            xfall = pers.tile([P, F * STRIP], BF16, tag="xfall", name="xfall")
            hbuf = [pers.tile([KC[c], NLOC], mlp_dt, tag=f"hbuf{c}", name=f"hbuf{c}") for c in range(5)]
            fsum = pers.tile([1, NLOC], F32, tag="fsum", name="fsum")
            def ztile(m):
                return zp.tile([MB[m], NLOC], mlp_dt, tag=f"zb{m}", name=f"zb{m}")
            st = [[stp.tile([MB[m], NG, 6], F32, tag=f"st{l}m{m}", name=f"st{l}m{m}") for m in range(4)]
                  for l in range(3)]
            ab_scale = [[stp.tile([MB[m], 1], F32, tag=f"av{l}m{m}", name=f"av{l}m{m}") for m in range(4)]
                        for l in range(3)]
            ab_shift = [[stp.tile([MB[m], 1], F32, tag=f"cv{l}m{m}", name=f"cv{l}m{m}") for m in range(4)]
                        for l in range(3)]

            # collective bounce buffers
            arin = [dramp.tile([HID, 2], F32, tag=f"arin{l}", name=f"arin{l}") for l in range(3)]
            arout = [dramp.tile([HID, 2], F32, tag=f"arout{l}", name=f"arout{l}") for l in range(3)]

            # ------- phase A: bulk gathers + slot-blend per (half, chunk) --
            def emit_half(h):
                f0 = 0
                for ch in CHUNKS:
                    G = gp.tile([P, 4 * HW_NIDX], BF16, tag="G", name="G")
                    for q in range(ch):
                        f = f0 + q
                        nc.gpsimd.dma_gather(
                            G[:, q * HW_NIDX:(q + 1) * HW_NIDX].rearrange(
                                "p (t w) -> p t w", w=RW),
                            embp[f * NPR:(f + 1) * NPR, :],
                            idx_t[:, f * P + h * (HW_NIDX // 16):
                                  f * P + (h + 1) * (HW_NIDX // 16)],
                            HW_NIDX, HW_NIDX, RW,
                        )
                    G4 = G[:, :ch * HW_NIDX].rearrange(
                        "p (c t w) -> p c t w", t=TPH, w=RW)

                    def wvk(k):
                        return (wv_t[:, k * FE + f0 * E: k * FE + (f0 + ch) * E]
                                .rearrange("p (c t) -> p c t", t=E)
                                [:, :, h * TPH:(h + 1) * TPH]
                                .unsqueeze(3).to_broadcast([P, ch, TPH, W17]))

                    hw = TPH * W17      # 136 cols per (field, half) strip part
                    acc = bp.tile([P, 4 * hw], BF16, tag="acc", name="acc")
                    tmp = bp.tile([P, 4 * hw], BF16, tag="tmp", name="tmp")
                    a4 = acc[:, :ch * hw].rearrange("p (c t w) -> p c t w", t=TPH, w=W17)
                    t4 = tmp[:, :ch * hw].rearrange("p (c t w) -> p c t w", t=TPH, w=W17)
                    nc.vector.tensor_tensor(out=a4, in0=G4[:, :, :, 0:17], in1=wvk(0), op=MUL)
                    nc.vector.tensor_tensor(out=t4, in0=G4[:, :, :, 17:34], in1=wvk(1), op=MUL)
                    nc.vector.tensor_tensor(out=a4, in0=a4, in1=t4, op=ADD)
                    nc.vector.tensor_tensor(out=t4, in0=G4[:, :, :, 34:51], in1=wvk(2), op=MUL)
                    nc.vector.tensor_tensor(out=a4, in0=a4, in1=t4, op=ADD)
                    nc.vector.tensor_tensor(out=t4, in0=G4[:, :, :, 51:68], in1=wvk(3), op=MUL)
                    # out: per-field strip slice [f*STRIP + h*hw : +hw], chunk-major
                    o4 = (xfall[:].rearrange("p (f s) -> p f s", s=STRIP)
                          [:, f0:f0 + ch, h * hw:(h + 1) * hw]
                          .rearrange("p c (t w) -> p c t w", w=W17))
                    nc.vector.tensor_tensor(out=o4, in0=a4, in1=t4, op=ADD)
                    f0 += ch

            # ---------------- phase B: per-tile norm + transposes ----------
            xf3 = xfall[:].rearrange("p (f s) -> p f s", s=STRIP)
            xfT = xfall[:].rearrange("p (f s) -> p s f", s=STRIP)

            def norm_tile(t):
                tcols = slice(t * P, (t + 1) * P)
                base = t * W17
                xe = xf3[:, :, base:base + E]
                sq = nrm.tile([P, FE], BF16, tag="sq", name="sq")
                nc.vector.tensor_tensor(out=sq[:], in0=xe, in1=xe, op=MUL)
                ss = nrm.tile([P, E], F32, tag="ss", name="ss")
                nc.vector.reduce_sum(out=ss[:], in_=sq[:].rearrange("p (f e) -> p e f", e=E),
                                     axis=mybir.AxisListType.X)
                sd = nrm.tile([P, E], F32, tag="sd", name="sd")
                nc.scalar.sqrt(out=sd[:], in_=ss[:])
                nc.vector.tensor_scalar_max(out=sd[:], in0=sd[:], scalar1=1e-12)
                inv = nrm.tile([P, E], F32, tag="inv", name="inv")
                nc.vector.reciprocal(out=inv[:], in_=sd[:])
                h = hp.tile([P, FE], BF16, tag="h", name="h")
                nc.vector.tensor_tensor(out=h[:], in0=xe,
                                        in1=inv[:].unsqueeze(1).to_broadcast([P, F, E]),
                                        op=MUL)
                s_t = nrm.tile([P, E], F32, tag="s", name="s")
                nc.vector.reduce_sum(out=s_t[:], in_=h[:].rearrange("p (f e) -> p e f", e=E),
                                     axis=mybir.AxisListType.X)
                # f2 = 0.5*(s^2 - ss*inv^2) summed over e; f1 = sum_f xs[.,16]
                qq = nrm.tile([P, E], F32, tag="qq", name="qq")
                nc.vector.tensor_mul(out=qq[:], in0=s_t[:], in1=s_t[:])
                iv2 = nrm.tile([P, E], F32, tag="iv2", name="iv2")
                nc.vector.tensor_mul(out=iv2[:], in0=inv[:], in1=inv[:])
                nc.vector.tensor_mul(out=iv2[:], in0=iv2[:], in1=ss[:])
                nc.vector.tensor_tensor(out=qq[:], in0=qq[:], in1=iv2[:], op=SUB)
                f2s = nrm.tile([P, 1], F32, tag="f2s", name="f2s")
                nc.vector.reduce_sum(out=f2s[:], in_=qq[:], axis=mybir.AxisListType.X)
                f1s = nrm.tile([P, 1], F32, tag="f1s", name="f1s")
                nc.vector.reduce_sum(out=f1s[:], in_=xfT[:, base + E:base + E + 1, :],
                                     axis=mybir.AxisListType.X)
                fm = nrm.tile([P, 1], mlp_dt, tag="fm", name="fm")
                nc.vector.scalar_tensor_tensor(out=fm[:], in0=f2s[:], scalar=0.5,
                                               in1=f1s[:], op0=MUL, op1=ADD)
                po = ps_po.tile([1, P], F32, tag="po", name="po")
                nc.tensor.matmul(po[:], fm[:], ident_t[:], start=True, stop=True)
                nc.vector.tensor_copy(out=fsum[:, tcols], in_=po[:])
                for c in range(5):
                    kc = KC[c]
                    tp = ps_t.tile([P, P], mlp_dt, tag="tps", name="tps")
                    nc.tensor.transpose(out=tp[:kc, :], in_=h[:, KO[c]:KO[c] + kc],
                                        identity=ident_t[:])
                    nc.scalar.copy(out=hbuf[c][:kc, tcols], in_=tp[:kc, :])

            def mlp_layer(l, g, src_get, wsb, kblocks, kofs):
                gcols = slice(g * S, (g + 1) * S)
                for m in range(4):
                    zps = ps_z.tile([P, S], F32, tag="zps", name="zps")
                    nkb = len(kblocks)
                    for kI in range(nkb):
                        nc.tensor.matmul(zps[:MB[m], :],
                                         wsb[kI][:, MO[m]:MO[m] + MB[m]],
                                         src_get(kI, gcols),
                                         start=(kI == 0), stop=(kI == nkb - 1))
                    nc.vector.bn_stats(out=st[l][m][:MB[m], g, :], in_=zps[:MB[m], :])
                    nc.vector.tensor_copy(out=zcur[m][:MB[m], gcols], in_=zps[:MB[m], :])

            def finalize_stats(l):
                for m in range(4):
                    mm = MB[m]
                    mv = nrm.tile([P, 2], F32, tag="mv", name="mv")
                    nc.vector.bn_aggr(out=mv[:mm, :], in_=st[l][m][:mm])
                    pk = nrm.tile([P, 2], F32, tag="pk", name="pk")
                    nc.vector.tensor_copy(out=pk[:mm, 0:1], in_=mv[:mm, 0:1])
                    tmp = nrm.tile([P, 1], F32, tag="tmp", name="tmp")
                    nc.vector.tensor_mul(out=tmp[:mm], in0=mv[:mm, 0:1], in1=mv[:mm, 0:1])
                    nc.vector.tensor_add(out=pk[:mm, 1:2], in0=mv[:mm, 1:2], in1=tmp[:mm])
                    nc.sync.dma_start(out=arin[l][MO[m]:MO[m] + mm, :], in_=pk[:mm, :])
                nc.gpsimd.collective_compute(
                    "AllReduce", mybir.AluOpType.add,
                    replica_groups=[list(range(n_cores))],
                    ins=[arin[l][:]], outs=[arout[l][:]],
                )
                for m in range(4):
                    mm = MB[m]
                    sm = nrm.tile([P, 2], F32, tag="sm", name="sm")
                    nc.sync.dma_start(out=sm[:mm, :], in_=arout[l][MO[m]:MO[m] + mm, :])
                    mu = nrm.tile([P, 1], F32, tag="mu", name="mu")
                    nc.vector.tensor_scalar_mul(out=mu[:mm], in0=sm[:mm, 0:1], scalar1=1.0 / n_cores)
                    e2 = nrm.tile([P, 1], F32, tag="e2", name="e2")
                    nc.vector.tensor_scalar_mul(out=e2[:mm], in0=sm[:mm, 1:2], scalar1=1.0 / n_cores)
                    var = nrm.tile([P, 1], F32, tag="var", name="var")
                    nc.vector.tensor_mul(out=var[:mm], in0=mu[:mm], in1=mu[:mm])
                    nc.vector.tensor_sub(out=var[:mm], in0=e2[:mm], in1=var[:mm])
                    sd = nrm.tile([P, 1], F32, tag="sdv", name="sdv")
                    nc.scalar.activation(out=sd[:mm], in_=var[:mm], func=AF.Sqrt,
                                         bias=eps_t[:mm], scale=1.0)
                    ri = nrm.tile([P, 1], F32, tag="ri", name="ri")
                    nc.vector.reciprocal(out=ri[:mm], in_=sd[:mm])
                    nc.vector.tensor_mul(out=ab_scale[l][m][:mm], in0=g_sb[l][m][:mm],
                                         in1=ri[:mm])
                    tmp2 = nrm.tile([P, 1], F32, tag="tmp2", name="tmp2")
                    nc.vector.tensor_mul(out=tmp2[:mm], in0=mu[:mm],
                                         in1=ab_scale[l][m][:mm])
                    nc.vector.tensor_sub(out=ab_shift[l][m][:mm], in0=bt_sb[l][m][:mm],
                                         in1=tmp2[:mm])

            # ============ emit program ============
            z1 = [ztile(m) for m in range(4)]
            zcur = z1
            emit_half(0)
            for t in range(TPH):
                norm_tile(t)
            # L1 for the first half overlaps the second half's gathers
            for g in range(NG // 2):
                mlp_layer(0, g,
                          lambda kI, gcols: hbuf[kI][:KC[kI], gcols],
                          w1sb, KC, KO)
            emit_half(1)
            for g in range(NG // 2, NG):
                for u in range(TPG):
                    norm_tile(g * TPG + u)
                mlp_layer(0, g,
                          lambda kI, gcols: hbuf[kI][:KC[kI], gcols],
                          w1sb, KC, KO)
            finalize_stats(0)

            z2 = [ztile(m) for m in range(4)]
            for g in range(NG):
                gcols = slice(g * S, (g + 1) * S)
                a1 = []
                for k in range(4):
                    at = actp.tile([MB[k], S], mlp_dt, tag=f"a{k}", name=f"a{k}")
                    nc.scalar.activation(out=at[:], in_=z1[k][:MB[k], gcols],
                                         func=AF.Relu, bias=ab_shift[0][k][:MB[k]],
                                         scale=ab_scale[0][k][:MB[k]])
                    a1.append(at)
                zcur = z2
                mlp_layer(1, g, lambda kI, gcols_: a1[kI][:], w2sb, MB, MO)
            finalize_stats(1)

            z3 = [ztile(m) for m in range(4)]
            for g in range(NG):
                gcols = slice(g * S, (g + 1) * S)
                a2 = []
                for k in range(4):
                    at = actp.tile([MB[k], S], mlp_dt, tag=f"b{k}", name=f"b{k}")
                    nc.scalar.activation(out=at[:], in_=z2[k][:MB[k], gcols],
                                         func=AF.Relu, bias=ab_shift[1][k][:MB[k]],
                                         scale=ab_scale[1][k][:MB[k]])
                    a2.append(at)
                zcur = z3
                mlp_layer(2, g, lambda kI, gcols_: a2[kI][:], w3sb, MB, MO)
            finalize_stats(2)

            for g in range(NG):
                gcols = slice(g * S, (g + 1) * S)
                po2 = ps_po.tile([1, S], F32, tag="po2", name="po2")
                for m in range(4):
                    at = actp.tile([MB[m], S], mlp_dt, tag=f"c{m}", name=f"c{m}")
                    nc.scalar.activation(out=at[:], in_=z3[m][:MB[m], gcols],
                                         func=AF.Relu, bias=ab_shift[2][m][:MB[m]],
                                         scale=ab_scale[2][m][:MB[m]])
                    nc.tensor.matmul(po2[:], ones_t[:MB[m], :], at[:],
                                     start=(m == 0), stop=(m == 3))
                ot = osb.tile([1, S], F32, tag="ot", name="ot")
                nc.vector.tensor_add(out=ot[:], in0=po2[:], in1=fsum[:, gcols])
                nc.vector.tensor_scalar(out=ot[:], in0=ot[:], scalar1=bias_t[:, :],
                                        scalar2=None, op0=mybir.AluOpType.add)
                nc.sync.dma_start(out=out[:, gcols], in_=ot[:])

    nc.compile()
    return nc


def _prep_shared(emb1, emb2, W1, W2, W3, g1, bt1, g2, bt2, g3, bt3, bias):
    ent = np.concatenate(
        [np.asarray(emb2, np.float32).reshape(F * V, E),
         np.asarray(emb1, np.float32).reshape(F * V, 1)], axis=1)
    entb = ent.astype(ml_dtypes.bfloat16)              # (F*V, 17)
    packed = np.zeros((F * NPR, RW), dtype=ml_dtypes.bfloat16)
    packed[:, :PK * W17] = entb.reshape(F * NPR, PK * W17)
    return {
        "embp": packed,
        "w1t": np.ascontiguousarray(np.asarray(W1, np.float32).T).astype(ml_dtypes.bfloat16),
        "w2t": np.ascontiguousarray(np.asarray(W2, np.float32).T).astype(ml_dtypes.bfloat16),
        "w3t": np.ascontiguousarray(np.asarray(W3, np.float32).T).astype(ml_dtypes.bfloat16),
        "g1": np.asarray(g1, np.float32).reshape(HID, 1),
        "bt1": np.asarray(bt1, np.float32).reshape(HID, 1),
        "g2": np.asarray(g2, np.float32).reshape(HID, 1),
        "bt2": np.asarray(bt2, np.float32).reshape(HID, 1),
        "g3": np.asarray(g3, np.float32).reshape(HID, 1),
        "bt3": np.asarray(bt3, np.float32).reshape(HID, 1),
        "bias": np.asarray(bias, np.float32).reshape(1, 1),
        "identb": np.eye(P, dtype=np.float32).astype(ml_dtypes.bfloat16),
        "onesb": np.ones((P, 1), np.float32).astype(ml_dtypes.bfloat16),
    }


def _prep_core(xi, xv):
    # xi/xv: (NLOC, F) for this core
    pidx = (xi >> 2).astype(np.int16)                  # (2048, 39) < 25000
    slot = (xi & 3).astype(np.int64)
    # idx tile [128, F*128]: [p, f*128+c] = pidx[c*16 + p%16, f]
    arr = pidx.reshape(P, 16, F).transpose(1, 2, 0)    # (r=16, f, c=128)
    idxt = np.ascontiguousarray(np.tile(arr, (8, 1, 1)).reshape(P, F * P))
    # wv tile [128, 4*FE]: [p, k*624 + f*16 + t] = Xv[t*128+p, f] * (slot==k)
    xv3 = xv.reshape(NT, P, F)
    sl3 = slot.reshape(NT, P, F)
    wvt = np.zeros((P, 4 * FE), dtype=ml_dtypes.bfloat16)
    for k in range(4):
        mk = np.where(sl3 == k, xv3, 0.0).transpose(1, 2, 0)  # (p, f, t)
        wvt[:, k * FE:(k + 1) * FE] = mk.reshape(P, FE).astype(ml_dtypes.bfloat16)
    return idxt, wvt


def _prep_inputs(Xi, Xv, emb1, emb2, W1, W2, W3, g1, bt1, g2, bt2, g3, bt3, bias):
    shared = _prep_shared(emb1, emb2, W1, W2, W3, g1, bt1, g2, bt2, g3, bt3, bias)
    xi_all = np.asarray(Xi).reshape(N, F).astype(np.int64)
    xv_all = np.asarray(Xv, np.float32)
    in_maps = []
    for c in range(NC):
        rows = slice(c * NLOC, (c + 1) * NLOC)
        idxt, wvt = _prep_core(xi_all[rows], xv_all[rows])
        m = dict(shared)
        m["idx16"] = idxt
        m["wv"] = wvt
        in_maps.append(m)
    return in_maps


_NC_CACHE = {}


def kernel(Xi, Xv, emb1, emb2, W1, b1, g1, bt1, W2, b2, g2, bt2, W3, b3, g3,
           bt3, bias, _trace=False, _tmpdir=None):
    # b1/b2/b3 cancel inside training-mode BatchNorm (z - mean(z) is
    # bias-invariant), so they are accepted but unused.
    if "nc" not in _NC_CACHE:
        _NC_CACHE["nc"] = build_kernel()
    nc = _NC_CACHE["nc"]
    in_maps = _prep_inputs(Xi, Xv, emb1, emb2, W1, W2, W3,
                           g1, bt1, g2, bt2, g3, bt3, bias)
    res = run_bass_kernel_spmd(nc, in_maps, core_ids=list(range(NC)),
                               trace=_trace, tmpdir=_tmpdir)
    outp = np.concatenate([res.results[c]["out"].reshape(NLOC) for c in range(NC)])
    kernel.last_exec_time_ns = res.exec_time_ns
    return outp
